# revision 4
# baseline (speedup 1.0000x reference)
"""Multi-head attention forward (B=2, N=2048, DIM=1024, H=16, D=64) on 8 TRN2
NeuronCores.

Sharding: 2-way data parallel over batch x 4-way tensor parallel over heads.
Core c: batch c//4, heads 4*(c%4) .. 4*(c%4)+3.

Per-core device kernel (all matmuls bf16, fp32 PSUM accumulation):
  1. QK projection into transposed layout qkT [feat(part), tok], head dims
     pre-permuted (even then odd per head) so RoPE pairs sit in partition
     blocks of 32.  RoPE fused behind each chunk (Tile deps are range-based).
  2. RoPE: rot = qk*cos_rep + swap(qk)*sin_signed; swap is a partition block
     swap (SBUF->SBUF DMA), sin's sign folded host-side.
  3. V projection into natural [tok(part), d] layout, scattered into per-pair
     lhsT tiles: even head [V | ones-col] (denominator rides row 64), odd
     head [ones-col | zeros | V] (denominator at row 0, V at rows 64:128 so
     the normalize multiply lands directly in attnT[64:128] -- no shift DMA).
  4. The whole kernel is scheduled as one flat stream of 128 (pair, strip,
     kj) slots paced by the scalar engine's exp (16.8M elems, the hard
     floor).  Per slot: exp(kj) -> [filler matmuls] -> st(kj+1) -> pv(kj).
     The next slot's S^T matmul is pre-issued before pv so the exp stream
     never waits at strip boundaries.  V-proj and QK-proj chunks stream as
     fillers inside early strips (only v(0..7)+qk(2,0)+qk(0,0) run before
     the first exp); output-proj chunks fill the late strips.
  5. Normalize per (pair, strip): denominator rows are broadcast across
     partitions with two K=1 matmuls into a PSUM tile, one 128-lane
     reciprocal_approx_fast, then two multiplies write attnT[0:64] (even)
     and attnT[64:128] (odd) straight from the PV banks.  No gpsimd
     broadcast, no partition-hop DMA.
  6. Output projection partial per strip: out_part[tok, 1024].
Host sums the 4 head-group partials per batch and adds the bias.
"""

import numpy as np
import ml_dtypes
from contextlib import ExitStack

import concourse.bass as bass
import concourse.mybir as mybir
import concourse.tile as tile
from concourse import bacc
from concourse import bass2jax

# problem constants (hardcoded per contract)
B, N, DIM, H, D = 2, 2048, 1024, 16, 64
HL = 4                      # heads per core
QKF = 2 * HL * D            # 512 qk features per core
VF = HL * D                 # 256 v features per core
SCALE = D ** -0.5
P = 128
KT = DIM // P               # 8 k tiles of the model dim
NKJ = N // P                # 16 key tiles
BF16 = mybir.dt.bfloat16
F32 = mybir.dt.float32
NPBF16 = ml_dtypes.bfloat16

_CACHE = {}


def _build_nc():
    nc = bacc.Bacc("TRN2", target_bir_lowering=False)

    xT = nc.declare_dram_parameter("xT", [DIM, N], BF16, isOutput=False)
    wqkT = nc.declare_dram_parameter("wqkT", [DIM, QKF], BF16, isOutput=False)
    wvT = nc.declare_dram_parameter("wvT", [DIM, VF], BF16, isOutput=False)
    wp = nc.declare_dram_parameter("wp", [VF, DIM], BF16, isOutput=False)
    cs = nc.declare_dram_parameter("cs", [P, N], BF16, isOutput=False)
    sn = nc.declare_dram_parameter("sn", [P, N], BF16, isOutput=False)
    out = nc.declare_dram_parameter("out", [N, DIM], F32, isOutput=True)

    xT_r = xT.rearrange("(k p) n -> p k n", p=P)
    wqkT_r = wqkT.rearrange("(k p) f -> p k f", p=P)
    wvT_r = wvT.rearrange("(k p) f -> p k f", p=P)
    wp_r = wp.rearrange("(k p) f -> p k f", p=P)
    out_r = out.rearrange("(m p) f -> m p f", p=P)

    with tile.TileContext(nc) as tc:
        with ExitStack() as ctx:
            singles = ctx.enter_context(tc.tile_pool(name="singles", bufs=1))
            # PSUM budget: st 2x[128,1024]=4 banks, pv 2x[128,512]=2, prj 2
            psum_st = ctx.enter_context(tc.tile_pool(name="psum_st", bufs=2, space="PSUM"))
            psum_pv = ctx.enter_context(tc.tile_pool(name="psum_pv", bufs=2, space="PSUM"))
            psum_prj = ctx.enter_context(tc.tile_pool(name="psum_prj", bufs=2, space="PSUM"))
            expp = ctx.enter_context(tc.tile_pool(name="expp", bufs=6))
            outp = ctx.enter_context(tc.tile_pool(name="outp", bufs=4))
            smallp = ctx.enter_context(tc.tile_pool(name="smallp", bufs=4))
            ropep = ctx.enter_context(tc.tile_pool(name="ropep", bufs=4))

            # ---- static loads (chunked per token block so compute starts early)
            xT_sb = singles.tile([P, KT, N], BF16, tag="xT_sb")
            wqkT_sb = singles.tile([P, KT, QKF], BF16, tag="wqkT_sb")
            wvT_sb = singles.tile([P, KT, VF], BF16, tag="wvT_sb")
            cs_sb = singles.tile([P, N], BF16, tag="cs_sb")
            sn_sb = singles.tile([P, N], BF16, tag="sn_sb")
            # critical path for the first exp: wqkT m=2,0 + cos/sin + xT tok 0-511
            for m in (2, 0):
                nc.sync.dma_start(out=wqkT_sb[:, :, m * P:(m + 1) * P],
                                  in_=wqkT_r[:, :, m * P:(m + 1) * P])
            nc.sync.dma_start(out=cs_sb[:, 0:512], in_=cs[:, 0:512])
            nc.sync.dma_start(out=sn_sb[:, 0:512], in_=sn[:, 0:512])
            nc.sync.dma_start(out=xT_sb[:, 0:4, 0:512], in_=xT_r[:, 0:4, 0:512])
            nc.sync.dma_start(out=xT_sb[:, 4:KT, 0:512], in_=xT_r[:, 4:KT, 0:512])
            # then what the prelude v chunks need
            nc.sync.dma_start(out=wvT_sb, in_=wvT_r)
            nc.sync.dma_start(out=xT_sb[:, :, 512:1024], in_=xT_r[:, :, 512:1024])
            for m in (3, 1):
                nc.sync.dma_start(out=wqkT_sb[:, :, m * P:(m + 1) * P],
                                  in_=wqkT_r[:, :, m * P:(m + 1) * P])
            for b in range(1, 4):
                sl = slice(b * 512, (b + 1) * 512)
                nc.sync.dma_start(out=cs_sb[:, sl], in_=cs[:, sl])
                nc.sync.dma_start(out=sn_sb[:, sl], in_=sn[:, sl])
                if b >= 2:
                    nc.sync.dma_start(out=xT_sb[:, :, sl], in_=xT_r[:, :, sl])
            wp_sb = singles.tile([P, VF // P, DIM], BF16, tag="wp_sb")
            nc.sync.dma_start(out=wp_sb, in_=wp_r)

            qk_rot = singles.tile([P, 4, N], BF16, tag="qk_rot")
            # even head of pair j: [V(0:64) | ones col 64] -> denom at row 64
            vones_e = singles.tile([P, 2, NKJ, D + 1], BF16, tag="vones_e")
            # odd head: [ones col 0 | zeros | V(64:128)] -> denom at row 0,
            # V rows 64:128 so normalize writes attnT[64:128] directly
            vones_o = singles.tile([P, 2, NKJ, P], BF16, tag="vones_o")
            attnT = singles.tile([P, VF // P, N], BF16, tag="attnT")
            ones_sb = singles.tile([P, 64], BF16, tag="ones_sb")

            nc.gpsimd.memset(vones_o[:, :, :, 1:D], 0.0)
            nc.vector.memset(vones_e[:, :, :, D:D + 1], 1.0)
            nc.vector.memset(vones_o[:, :, :, 0:1], 1.0)
            nc.vector.memset(ones_sb, 1.0)

            # ---- QK projection chunk + fused RoPE -----------------------------
            open_qk = {}

            def qk_chunk(m, t, ks=None):
                sl = slice(t * 512, (t + 1) * 512)
                if ks is None or ks.start == 0:
                    ps = psum_prj.tile([P, 512], F32, tag="prj",
                                       name=f"qk_{m}_{t}")
                    if ks is not None:
                        open_qk[(m, t)] = ps
                else:
                    ps = open_qk.pop((m, t))
                for k in (range(KT) if ks is None else range(ks.start, ks.stop)):
                    nc.tensor.matmul(
                        ps,
                        lhsT=wqkT_sb[:, k, m * P:(m + 1) * P],
                        rhs=xT_sb[:, k, sl],
                        start=(k == 0),
                        stop=(k == KT - 1),
                    )
                if ks is not None and ks.stop != KT:
                    return
                raw = ropep.tile([P, 512], BF16, tag="raw")
                nc.vector.tensor_copy(raw, ps)
                sw = ropep.tile([P, 512], BF16, tag="sw")
                for a in range(0, P, 64):
                    nc.sync.dma_start(out=sw[a:a + 32, :], in_=raw[a + 32:a + 64, :])
                    nc.sync.dma_start(out=sw[a + 32:a + 64, :], in_=raw[a:a + 32, :])
                t1 = ropep.tile([P, 512], BF16, tag="t1")
                nc.vector.tensor_mul(t1, raw, cs_sb[:, sl])
                t2 = ropep.tile([P, 512], BF16, tag="t2")
                nc.vector.tensor_mul(t2, sw, sn_sb[:, sl])
                nc.vector.tensor_add(qk_rot[:, m, sl], t1, t2)

            # ---- V projection chunk: scatter into vones_e / vones_o -----------
            open_v = {}

            def v_chunk(t, ks=None):
                if ks is None or ks.start == 0:
                    ps = psum_prj.tile([P, 8, 64], F32, tag="prj",
                                       name=f"v_{t}")
                    if ks is not None:
                        open_v[t] = ps
                else:
                    ps = open_v.pop(t)
                for k in (range(KT) if ks is None else range(ks.start, ks.stop)):
                    nc.tensor.matmul(
                        ps[:, 0:4, :],
                        lhsT=xT_sb[:, k, t * P:(t + 1) * P],
                        rhs=wvT_sb[:, k, :],
                        start=(k == 0),
                        stop=(k == KT - 1),
                    )
                if ks is not None and ks.stop != KT:
                    return
                # wvT cols are host-permuted [h0, h2, h1, h3]
                nc.vector.tensor_copy(vones_e[:, :, t, 0:D], ps[:, 0:2, :])
                nc.vector.tensor_copy(vones_o[:, :, t, D:2 * D], ps[:, 2:4, :])

            # ---- output projection chunk --------------------------------------
            def proj_chunk(mt, ch, last=False):
                if last:
                    ps = psum_st.tile([P, 1024], F32, tag="st",
                                      name=f"prj_{mt}_{ch}")[:, 0:512]
                else:
                    ps = psum_prj.tile([P, 512], F32, tag="prj",
                                       name=f"prj_{mt}_{ch}")
                for kt in range(VF // P):
                    nc.tensor.matmul(
                        ps,
                        lhsT=attnT[:, kt, mt * P:(mt + 1) * P],
                        rhs=wp_sb[:, kt, ch * 512:(ch + 1) * 512],
                        start=(kt == 0),
                        stop=(kt == VF // P - 1),
                    )
                ob = outp.tile([P, 512], F32, tag="ob")
                if last and (mt + ch) % 2 == 0:
                    nc.scalar.copy(ob, ps)
                else:
                    nc.vector.tensor_copy(ob, ps)
                nc.sync.dma_start(out=out_r[mt, :, ch * 512:(ch + 1) * 512], in_=ob)

            # ---- normalize a (pair, strip): matmul-broadcast the denominators,
            # one full-width reciprocal, two multiplies straight into attnT.
            def norm_pair(j, s, pv_e, pv_o):
                sl = slice(s * 512, (s + 1) * 512)
                den = smallp.tile([P, 512], BF16, tag="den")
                nc.vector.tensor_copy(den[64:65, :], pv_e[64:65, :])
                nc.vector.tensor_copy(den[0:1, :], pv_o[0:1, :])
                R = psum_prj.tile([P, 512], F32, tag="prj", name=f"R_{j}_{s}")
                nc.tensor.matmul(R[0:64, :], lhsT=ones_sb[64:65, 0:64],
                                 rhs=den[64:65, :], start=True, stop=True,
                                 tile_position=(64, 0))
                nc.tensor.matmul(R[64:128, :], lhsT=ones_sb[0:1, 0:64],
                                 rhs=den[0:1, :], start=True, stop=True,
                                 tile_position=(0, 64))
                Rr = smallp.tile([P, 512], F32, tag="Rr")
                nc.vector.reciprocal_approx_fast(out=Rr, in_=R)
                nc.vector.tensor_mul(attnT[0:64, j, sl], pv_e[0:64, :],
                                     Rr[0:64, :])
                nc.vector.tensor_mul(attnT[64:128, j, sl], pv_o[64:128, :],
                                     Rr[64:128, :])

            # ---- filler schedule: key (j, s, kj) ------------------------------
            H0, H1 = slice(0, 4), slice(4, KT)
            fillers = {}
            # strip (0,0): v(8..15) halves + remaining pair-0 k chunks + q(0,1)
            for i in range(8):
                fillers[(0, 0, 2 * i)] = [("v", 8 + i, H0)]
                fillers[(0, 0, 2 * i + 1)] = [("v", 8 + i, H1)]
            for m, t, kj0, kj1 in ((2, 1, 1, 2), (2, 2, 5, 6), (2, 3, 9, 10),
                                   (0, 1, 12, 13)):
                fillers[(0, 0, kj0)].append(("qk", m, t, H0))
                fillers[(0, 0, kj1)].append(("qk", m, t, H1))
            # strips (0,1)/(0,2): stream pair-1 prep + remaining q chunks
            for s, specs in ((1, ((0, 2), (3, 0), (3, 1), (1, 0))),
                             (2, ((0, 3), (3, 2), (3, 3), (1, 1)))):
                for i, (m, t) in enumerate(specs):
                    fillers.setdefault((0, s, 4 * i), []).append(("qk", m, t, H0))
                    fillers.setdefault((0, s, 4 * i + 2), []).append(("qk", m, t, H1))
            for i, (m, t) in enumerate(((1, 2), (1, 3))):
                fillers.setdefault((0, 3, 4 * i), []).append(("qk", m, t, H0))
                fillers.setdefault((0, 3, 4 * i + 2), []).append(("qk", m, t, H1))
            # strips (1,1..3): output projection for the previous strip
            for s in (1, 2, 3):
                for i in range(8):
                    fillers.setdefault((1, s, 2 * i + 1), []).append(
                        ("proj", 4 * (s - 1) + i // 2, i % 2))

            def run_filler(f):
                if f[0] == "v":
                    v_chunk(f[1], f[2])
                elif f[0] == "qk":
                    qk_chunk(f[1], f[2], f[3])
                else:
                    proj_chunk(f[1], f[2])

            # ---- prelude ------------------------------------------------------
            qk_chunk(2, 0)
            qk_chunk(0, 0)
            for t in range(8):
                v_chunk(t)

            # ---- main stream: 128 slots, exp-paced ----------------------------
            slots = [(j, s, kj) for j in range(2) for s in range(4)
                     for kj in range(NKJ)]

            def emit_st(j, s, kj):
                st = psum_st.tile([P, 1024], F32, tag="st",
                                  name=f"st_{j}_{s}_{kj}")
                nc.tensor.matmul(
                    st[:, 0:512],
                    lhsT=qk_rot[0:64, 2 + j, kj * P:(kj + 1) * P],
                    rhs=qk_rot[0:64, j, s * 512:(s + 1) * 512],
                    start=True, stop=True,
                    tile_position=(0, 0),
                )
                nc.tensor.matmul(
                    st[:, 512:1024],
                    lhsT=qk_rot[64:P, 2 + j, kj * P:(kj + 1) * P],
                    rhs=qk_rot[64:P, j, s * 512:(s + 1) * 512],
                    start=True, stop=True,
                    tile_position=(64, 0),
                )
                return st

            st_tiles = {slots[0]: emit_st(*slots[0])}
            pv_cur = None
            for i, (j, s, kj) in enumerate(slots):
                st = st_tiles.pop((j, s, kj))
                es = expp.tile([P, 1024], BF16, tag="expS")
                nc.scalar.activation(
                    es, st, mybir.ActivationFunctionType.Exp, scale=SCALE
                )
                for f in fillers.get((j, s, kj), ()):
                    run_filler(f)
                if i + 1 < len(slots):
                    st_tiles[slots[i + 1]] = emit_st(*slots[i + 1])
                if kj == 0:
                    pv_cur = (
                        psum_pv.tile([P, 512], F32, tag="pv", name=f"pve_{j}_{s}"),
                        psum_pv.tile([P, 512], F32, tag="pv", name=f"pvo_{j}_{s}"),
                    )
                nc.tensor.matmul(
                    pv_cur[0][0:D + 1, :],
                    lhsT=vones_e[:, j, kj, :],
                    rhs=es[:, 0:512],
                    start=(kj == 0), stop=(kj == NKJ - 1),
                )
                nc.tensor.matmul(
                    pv_cur[1],
                    lhsT=vones_o[:, j, kj, :],
                    rhs=es[:, 512:1024],
                    start=(kj == 0), stop=(kj == NKJ - 1),
                )
                if kj == NKJ - 1:
                    norm_pair(j, s, *pv_cur)

            # tail: last strip's projection (st pool is free by now)
            for mt in range(12, 16):
                for ch in range(2):
                    proj_chunk(mt, ch, last=True)

    nc.compile()
    return nc


def _make_in_maps(x, freqs, w_qkv, w_proj):
    # RoPE even/odd permutation of q/k head dims (host side, free)
    evens = np.arange(0, D, 2)
    odds = np.arange(1, D, 2)
    perm64 = np.concatenate([evens, odds])
    permH = np.concatenate([h * D + perm64 for h in range(HL)])
    # v columns regrouped [h0, h2, h1, h3] so even/odd scatter is 2 copies
    permV = np.concatenate([np.arange(h * D, (h + 1) * D)
                            for h in (0, 2, 1, 3)])

    wq = w_qkv[0:DIM]
    wk = w_qkv[DIM:2 * DIM]
    wv = w_qkv[2 * DIM:3 * DIM]

    cos = np.cos(freqs).astype(np.float32)   # [N, 32]
    sin = np.sin(freqs).astype(np.float32)
    pidx = np.arange(P) % 32
    cs_rep = cos[:, pidx].T.copy()           # [128, N]
    sgn = np.where((np.arange(P) % 64) < 32, -1.0, 1.0).astype(np.float32)
    sn_rep = (sin[:, pidx] * sgn[None, :]).T.copy()
    cs_b = cs_rep.astype(NPBF16)
    sn_b = sn_rep.astype(NPBF16)

    in_maps = []
    for c in range(8):
        b, g = c // 4, c % 4
        rows = slice(g * VF, (g + 1) * VF)
        wq_p = wq[rows][permH]               # [256, 1024]
        wk_p = wk[rows][permH]
        wqkT = np.concatenate([wq_p, wk_p], axis=0).T.copy()   # [1024, 512]
        wvT = wv[rows][permV].T.copy()                         # [1024, 256]
        wp_rhs = w_proj[:, rows].T.copy()                      # [256, 1024]
        xT = x[b].T.copy()                                     # [1024, 2048]
        in_maps.append({
            "xT": xT.astype(NPBF16),
            "wqkT": wqkT.astype(NPBF16),
            "wvT": wvT.astype(NPBF16),
            "wp": wp_rhs.astype(NPBF16),
            "cs": cs_b,
            "sn": sn_b,
        })
    return in_maps


def _reset_device():
    try:
        import ctypes
        import jax
        jax.devices()
        lib = ctypes.CDLL("/opt/axon/libaxon_pjrt.so")
        if hasattr(lib, "axon_reset"):
            lib.axon_reset.restype = ctypes.c_int64
            lib.axon_reset()
    except Exception:
        pass


def _get_nc():
    if "nc" not in _CACHE:
        _CACHE["nc"] = _build_nc()
    return _CACHE["nc"]


def kernel(x, freqs, w_qkv, w_proj, b_proj):
    x = np.asarray(x, dtype=np.float32)
    freqs = np.asarray(freqs, dtype=np.float32)
    w_qkv = np.asarray(w_qkv, dtype=np.float32)
    w_proj = np.asarray(w_proj, dtype=np.float32)
    b_proj = np.asarray(b_proj, dtype=np.float32)

    nc = _get_nc()
    in_maps = _make_in_maps(x, freqs, w_qkv, w_proj)
    try:
        results = bass2jax.run_bass_via_pjrt(nc, in_maps, n_cores=8)
    except Exception:
        # a previously crashed run can leave the accelerator unrecoverable;
        # reset once and retry
        _reset_device()
        results = bass2jax.run_bass_via_pjrt(nc, in_maps, n_cores=8)

    out = np.zeros((B, N, DIM), dtype=np.float32)
    for c in range(8):
        out[c // 4] += np.asarray(results[c]["out"], dtype=np.float32)
    out += b_proj[None, None, :]
    return out


# revision 14
# speedup vs baseline: 1.0190x; 1.0190x over previous
"""Multi-head attention forward (B=2, N=2048, DIM=1024, H=16, D=64) on 8 TRN2
NeuronCores.

Sharding: 2-way data parallel over batch x 4-way tensor parallel over heads.
Core c: batch c//4, heads 4*(c%4) .. 4*(c%4)+3.

Per-core device kernel (all matmuls bf16, fp32 PSUM accumulation):
  1. QK projection into transposed layout qkT [feat(part), tok], head dims
     pre-permuted (even then odd per head) so RoPE pairs sit in partition
     blocks of 32.  RoPE fused behind each chunk (Tile deps are range-based).
  2. RoPE: rot = qk*cos_rep + swap(qk)*sin_signed; swap is a partition block
     swap (SBUF->SBUF DMA), sin's sign folded host-side.
  3. V projection into natural [tok(part), d] layout, scattered into per-pair
     lhsT tiles: even head [V | ones-col] (denominator rides row 64), odd
     head [ones-col | zeros | V] (denominator at row 0, V at rows 64:128 so
     the normalize multiply lands directly in attnT[64:128] -- no shift DMA).
  4. The whole kernel is scheduled as one flat stream of 128 (pair, strip,
     kj) slots paced by the scalar engine's exp (16.8M elems, the hard
     floor).  Per slot: exp(kj) -> [filler matmuls] -> st(kj+1) -> pv(kj).
     The next slot's S^T matmul is pre-issued before pv so the exp stream
     never waits at strip boundaries.  V-proj and QK-proj chunks stream as
     fillers inside early strips (only v(0..7)+qk(2,0)+qk(0,0) run before
     the first exp); output-proj chunks fill the late strips.
  5. Normalize per (pair, strip): denominator rows are broadcast across
     partitions with two K=1 matmuls into a PSUM tile, one 128-lane
     reciprocal_approx_fast, then two multiplies write attnT[0:64] (even)
     and attnT[64:128] (odd) straight from the PV banks.  No gpsimd
     broadcast, no partition-hop DMA.
  6. Output projection partial per strip: out_part[tok, 1024].
Host sums the 4 head-group partials per batch and adds the bias.
"""

import numpy as np
import ml_dtypes
from contextlib import ExitStack

import concourse.bass as bass
import concourse.mybir as mybir
import concourse.tile as tile
from concourse import bacc
from concourse import bass2jax

# problem constants (hardcoded per contract)
B, N, DIM, H, D = 2, 2048, 1024, 16, 64
HL = 4                      # heads per core
QKF = 2 * HL * D            # 512 qk features per core
VF = HL * D                 # 256 v features per core
SCALE = D ** -0.5
P = 128
KT = DIM // P               # 8 k tiles of the model dim
NKJ = N // P                # 16 key tiles
BF16 = mybir.dt.bfloat16
F32 = mybir.dt.float32
NPBF16 = ml_dtypes.bfloat16

_CACHE = {}


def _build_nc():
    nc = bacc.Bacc("TRN2", target_bir_lowering=False)

    xT = nc.declare_dram_parameter("xT", [DIM, N], BF16, isOutput=False)
    wqkT = nc.declare_dram_parameter("wqkT", [DIM, QKF], BF16, isOutput=False)
    wvT = nc.declare_dram_parameter("wvT", [DIM, VF], BF16, isOutput=False)
    wp = nc.declare_dram_parameter("wp", [VF, DIM], BF16, isOutput=False)
    cs = nc.declare_dram_parameter("cs", [P, N], BF16, isOutput=False)
    sn = nc.declare_dram_parameter("sn", [P, N], BF16, isOutput=False)
    out = nc.declare_dram_parameter("out", [N, DIM], F32, isOutput=True)

    xT_r = xT.rearrange("(k p) n -> p k n", p=P)
    wqkT_r = wqkT.rearrange("(k p) f -> p k f", p=P)
    wvT_r = wvT.rearrange("(k p) f -> p k f", p=P)
    wp_r = wp.rearrange("(k p) f -> p k f", p=P)
    out_r = out.rearrange("(m p) f -> m p f", p=P)

    with tile.TileContext(nc) as tc:
        with ExitStack() as ctx:
            singles = ctx.enter_context(tc.tile_pool(name="singles", bufs=1))
            # PSUM budget: st 2x[128,1024]=4 banks, pv 2x[128,512]=2, prj 2
            psum_st = ctx.enter_context(tc.tile_pool(name="psum_st", bufs=2, space="PSUM"))
            psum_pv = ctx.enter_context(tc.tile_pool(name="psum_pv", bufs=2, space="PSUM"))
            psum_prj = ctx.enter_context(tc.tile_pool(name="psum_prj", bufs=2, space="PSUM"))
            expp = ctx.enter_context(tc.tile_pool(name="expp", bufs=8))
            outp = ctx.enter_context(tc.tile_pool(name="outp", bufs=4))
            smallp = ctx.enter_context(tc.tile_pool(name="smallp", bufs=4))
            ropep = ctx.enter_context(tc.tile_pool(name="ropep", bufs=4))

            # ---- static loads (chunked per token block so compute starts early)
            xT_sb = singles.tile([P, KT, N], BF16, tag="xT_sb")
            wqkT_sb = singles.tile([P, KT, QKF], BF16, tag="wqkT_sb")
            wvT_sb = singles.tile([P, KT, VF], BF16, tag="wvT_sb")
            cs_sb = singles.tile([P, N], BF16, tag="cs_sb")
            sn_sb = singles.tile([P, N], BF16, tag="sn_sb")
            # critical path for the first exp: wqkT m=2,0 + cos/sin + xT tok
            # 0-511, then wvT for the prelude v chunks
            for m in (2, 0):
                nc.sync.dma_start(out=wqkT_sb[:, :, m * P:(m + 1) * P],
                                  in_=wqkT_r[:, :, m * P:(m + 1) * P])
            nc.sync.dma_start(out=cs_sb[:, 0:512], in_=cs[:, 0:512])
            nc.sync.dma_start(out=sn_sb[:, 0:512], in_=sn[:, 0:512])
            nc.sync.dma_start(out=xT_sb[:, 0:4, 0:512], in_=xT_r[:, 0:4, 0:512])
            nc.sync.dma_start(out=xT_sb[:, 4:KT, 0:512], in_=xT_r[:, 4:KT, 0:512])
            nc.sync.dma_start(out=wvT_sb, in_=wvT_r)
            nc.sync.dma_start(out=xT_sb[:, :, 512:1024], in_=xT_r[:, :, 512:1024])
            for m in (3, 1):
                nc.sync.dma_start(out=wqkT_sb[:, :, m * P:(m + 1) * P],
                                  in_=wqkT_r[:, :, m * P:(m + 1) * P])
            for b in range(1, 4):
                sl = slice(b * 512, (b + 1) * 512)
                nc.sync.dma_start(out=cs_sb[:, sl], in_=cs[:, sl])
                nc.sync.dma_start(out=sn_sb[:, sl], in_=sn[:, sl])
                if b >= 2:
                    nc.sync.dma_start(out=xT_sb[:, :, sl], in_=xT_r[:, :, sl])
            wp_sb = singles.tile([P, VF // P, DIM], BF16, tag="wp_sb")
            nc.sync.dma_start(out=wp_sb, in_=wp_r)

            qk_rot = singles.tile([P, 4, N], BF16, tag="qk_rot")
            # even head of pair j: [V(0:64) | ones col 64] -> denom at row 64
            vones_e = singles.tile([P, 2, NKJ, D + 1], BF16, tag="vones_e")
            # odd head: [ones col 0 | zeros | V(64:128)] -> denom at row 0,
            # V rows 64:128 so normalize writes attnT[64:128] directly
            vones_o = singles.tile([P, 2, NKJ, P], BF16, tag="vones_o")
            attnT = singles.tile([P, VF // P, N], BF16, tag="attnT")
            ones_sb = singles.tile([P, 64], BF16, tag="ones_sb")

            nc.gpsimd.memset(vones_o[:, :, :, 1:D], 0.0)
            nc.vector.memset(vones_e[:, :, :, D:D + 1], 1.0)
            nc.vector.memset(vones_o[:, :, :, 0:1], 1.0)
            nc.vector.memset(ones_sb, 1.0)

            # ---- QK projection chunk + fused RoPE -----------------------------
            open_qk = {}

            def qk_chunk(m, t, ks=None):
                sl = slice(t * 512, (t + 1) * 512)
                if ks is None or ks.start == 0:
                    ps = psum_prj.tile([P, 512], F32, tag="prj",
                                       name=f"qk_{m}_{t}")
                    if ks is not None:
                        open_qk[(m, t)] = ps
                else:
                    ps = open_qk.pop((m, t))
                for k in (range(KT) if ks is None else range(ks.start, ks.stop)):
                    nc.tensor.matmul(
                        ps,
                        lhsT=wqkT_sb[:, k, m * P:(m + 1) * P],
                        rhs=xT_sb[:, k, sl],
                        start=(k == 0),
                        stop=(k == KT - 1),
                    )
                if ks is not None and ks.stop != KT:
                    return
                raw = ropep.tile([P, 512], BF16, tag="raw")
                nc.vector.tensor_copy(raw, ps)
                sw = ropep.tile([P, 512], BF16, tag="sw")
                for a in range(0, P, 64):
                    nc.sync.dma_start(out=sw[a:a + 32, :], in_=raw[a + 32:a + 64, :])
                    nc.sync.dma_start(out=sw[a + 32:a + 64, :], in_=raw[a:a + 32, :])
                t1 = ropep.tile([P, 512], BF16, tag="t1")
                nc.vector.tensor_mul(t1, raw, cs_sb[:, sl])
                t2 = ropep.tile([P, 512], BF16, tag="t2")
                nc.vector.tensor_mul(t2, sw, sn_sb[:, sl])
                nc.vector.tensor_add(qk_rot[:, m, sl], t1, t2)

            # ---- V projection chunk: scatter into vones_e / vones_o -----------
            open_v = {}

            def v_chunk(t, ks=None):
                if ks is None or ks.start == 0:
                    ps = psum_prj.tile([P, 8, 64], F32, tag="prj",
                                       name=f"v_{t}")
                    if ks is not None:
                        open_v[t] = ps
                else:
                    ps = open_v.pop(t)
                for k in (range(KT) if ks is None else range(ks.start, ks.stop)):
                    nc.tensor.matmul(
                        ps[:, 0:4, :],
                        lhsT=xT_sb[:, k, t * P:(t + 1) * P],
                        rhs=wvT_sb[:, k, :],
                        start=(k == 0),
                        stop=(k == KT - 1),
                    )
                if ks is not None and ks.stop != KT:
                    return
                # wvT cols are host-permuted [h0, h2, h1, h3]
                nc.vector.tensor_copy(vones_e[:, :, t, 0:D], ps[:, 0:2, :])
                nc.vector.tensor_copy(vones_o[:, :, t, D:2 * D], ps[:, 2:4, :])

            # ---- output projection chunk --------------------------------------
            def proj_chunk(mt, ch, last=False):
                idx = 2 * mt + ch
                if last and idx % 2 == 0:
                    ps = psum_st.tile([P, 1024], F32, tag="st",
                                      name=f"prj_{mt}_{ch}")[:, 0:512]
                else:
                    ps = psum_prj.tile([P, 512], F32, tag="prj",
                                       name=f"prj_{mt}_{ch}")
                for kt in range(VF // P):
                    nc.tensor.matmul(
                        ps,
                        lhsT=attnT[:, kt, mt * P:(mt + 1) * P],
                        rhs=wp_sb[:, kt, ch * 512:(ch + 1) * 512],
                        start=(kt == 0),
                        stop=(kt == VF // P - 1),
                    )
                ob = outp.tile([P, 512], F32, tag="ob")
                if last and idx % 2 == 0:
                    nc.scalar.copy(ob, ps)
                else:
                    nc.vector.tensor_copy(ob, ps)
                nc.sync.dma_start(out=out_r[mt, :, ch * 512:(ch + 1) * 512], in_=ob)

            # ---- normalize a (pair, strip): matmul-broadcast the denominators,
            # one full-width reciprocal, two multiplies straight into attnT.
            def norm_pair(j, s, pv_e, pv_o):
                sl = slice(s * 512, (s + 1) * 512)
                den = smallp.tile([P, 512], BF16, tag="den")
                nc.vector.tensor_copy(den[64:65, :], pv_e[64:65, :])
                nc.vector.tensor_copy(den[0:1, :], pv_o[0:1, :])
                # one accumulation group: only the first matmul clears the
                # bank's has_written bits (a second start=True clear could
                # race with the first matmul's writes)
                R = psum_prj.tile([P, 512], F32, tag="prj", name=f"R_{j}_{s}")
                nc.tensor.matmul(R[0:64, :], lhsT=ones_sb[64:65, 0:64],
                                 rhs=den[64:65, :], start=True, stop=False,
                                 tile_position=(64, 0), skip_group_check=True)
                nc.tensor.matmul(R[64:128, :], lhsT=ones_sb[0:1, 0:64],
                                 rhs=den[0:1, :], start=False, stop=True,
                                 tile_position=(0, 64), skip_group_check=True)
                Rr = smallp.tile([P, 512], F32, tag="Rr")
                nc.vector.reciprocal_approx_fast(out=Rr, in_=R)
                nc.vector.tensor_mul(attnT[0:64, j, sl], pv_e[0:64, :],
                                     Rr[0:64, :])
                nc.vector.tensor_mul(attnT[64:128, j, sl], pv_o[64:128, :],
                                     Rr[64:128, :])

            # ---- filler schedule: key (j, s, kj) ------------------------------
            H0, H1 = slice(0, 4), slice(4, KT)
            fillers = {}
            # strip (0,0): v(4..15) stream + remaining pair-0 k chunks + q(0,1)
            for kj in range(12):
                fillers[(0, 0, kj)] = [("v", kj + 4, None)]
            for m, t, kj0, kj1 in ((2, 1, 1, 2), (2, 2, 5, 6), (2, 3, 9, 10),
                                   (0, 1, 12, 13)):
                fillers.setdefault((0, 0, kj0), []).append(("qk", m, t, H0))
                fillers.setdefault((0, 0, kj1), []).append(("qk", m, t, H1))
            # strips (0,1)/(0,2): stream pair-1 prep + remaining q chunks
            for s, specs in ((1, ((0, 2), (3, 0), (3, 1), (1, 0))),
                             (2, ((0, 3), (3, 2), (3, 3), (1, 1)))):
                for i, (m, t) in enumerate(specs):
                    fillers.setdefault((0, s, 4 * i), []).append(("qk", m, t, H0))
                    fillers.setdefault((0, s, 4 * i + 2), []).append(("qk", m, t, H1))
            for i, (m, t) in enumerate(((1, 2), (1, 3))):
                fillers.setdefault((0, 3, 4 * i), []).append(("qk", m, t, H0))
                fillers.setdefault((0, 3, 4 * i + 2), []).append(("qk", m, t, H1))
            # strips (1,1..3): output projection for the previous strip
            for s in (1, 2, 3):
                for i, kj in enumerate((3, 4, 6, 7, 9, 10, 12, 13)):
                    fillers.setdefault((1, s, kj), []).append(
                        ("proj", 4 * (s - 1) + i // 2, i % 2))

            def run_filler(f):
                if f[0] == "v":
                    v_chunk(f[1], f[2])
                elif f[0] == "qk":
                    qk_chunk(f[1], f[2], f[3])
                else:
                    proj_chunk(f[1], f[2])

            # ---- main stream: 128 slots, exp-paced ----------------------------
            slots = [(j, s, kj) for j in range(2) for s in range(4)
                     for kj in range(NKJ)]

            def emit_st(j, s, kj):
                st = psum_st.tile([P, 1024], F32, tag="st",
                                  name=f"st_{j}_{s}_{kj}")
                nc.tensor.matmul(
                    st[:, 0:512],
                    lhsT=qk_rot[0:64, 2 + j, kj * P:(kj + 1) * P],
                    rhs=qk_rot[0:64, j, s * 512:(s + 1) * 512],
                    start=True, stop=True,
                    tile_position=(0, 0),
                )
                nc.tensor.matmul(
                    st[:, 512:1024],
                    lhsT=qk_rot[64:P, 2 + j, kj * P:(kj + 1) * P],
                    rhs=qk_rot[64:P, j, s * 512:(s + 1) * 512],
                    start=True, stop=True,
                    tile_position=(64, 0),
                )
                return st

            # ---- prelude: first two qk chunks feed st(0)/st(1) so the exp
            # stream starts as early as possible; v(0..3) fill behind them
            qk_chunk(2, 0)
            qk_chunk(0, 0)
            st_tiles = {slots[0]: emit_st(*slots[0]),
                        slots[1]: emit_st(*slots[1])}
            for t in range(4):
                v_chunk(t)
            pv_cur = None
            for i, (j, s, kj) in enumerate(slots):
                st = st_tiles.pop((j, s, kj))
                es = expp.tile([P, 1024], BF16, tag="expS")
                nc.scalar.activation(
                    es, st, mybir.ActivationFunctionType.Exp, scale=SCALE
                )
                # st lookahead: 1 mid-strip, 2 entering a strip boundary so the
                # exp stream never waits behind the norm-gated pv of slot 0
                if 1 <= kj <= 14 and i + 1 < len(slots):
                    st_tiles[slots[i + 1]] = emit_st(*slots[i + 1])
                for f in fillers.get((j, s, kj), ()):
                    run_filler(f)
                if kj >= 14 and i + 2 < len(slots):
                    st_tiles[slots[i + 2]] = emit_st(*slots[i + 2])
                if kj == 0:
                    pv_cur = (
                        psum_pv.tile([P, 512], F32, tag="pv", name=f"pve_{j}_{s}"),
                        psum_pv.tile([P, 512], F32, tag="pv", name=f"pvo_{j}_{s}"),
                    )
                nc.tensor.matmul(
                    pv_cur[0][0:D + 1, :],
                    lhsT=vones_e[:, j, kj, :],
                    rhs=es[:, 0:512],
                    start=(kj == 0), stop=(kj == NKJ - 1),
                )
                nc.tensor.matmul(
                    pv_cur[1],
                    lhsT=vones_o[:, j, kj, :],
                    rhs=es[:, 512:1024],
                    start=(kj == 0), stop=(kj == NKJ - 1),
                )
                if kj == NKJ - 1:
                    norm_pair(j, s, *pv_cur)

            # tail: last strip's projection (st pool is free by now)
            for mt in range(12, 16):
                for ch in range(2):
                    proj_chunk(mt, ch, last=True)

    nc.compile()
    return nc


def _make_in_maps(x, freqs, w_qkv, w_proj):
    # RoPE even/odd permutation of q/k head dims (host side, free)
    evens = np.arange(0, D, 2)
    odds = np.arange(1, D, 2)
    perm64 = np.concatenate([evens, odds])
    permH = np.concatenate([h * D + perm64 for h in range(HL)])
    # v columns regrouped [h0, h2, h1, h3] so even/odd scatter is 2 copies
    permV = np.concatenate([np.arange(h * D, (h + 1) * D)
                            for h in (0, 2, 1, 3)])

    wq = w_qkv[0:DIM]
    wk = w_qkv[DIM:2 * DIM]
    wv = w_qkv[2 * DIM:3 * DIM]

    cos = np.cos(freqs).astype(np.float32)   # [N, 32]
    sin = np.sin(freqs).astype(np.float32)
    pidx = np.arange(P) % 32
    cs_rep = cos[:, pidx].T.copy()           # [128, N]
    sgn = np.where((np.arange(P) % 64) < 32, -1.0, 1.0).astype(np.float32)
    sn_rep = (sin[:, pidx] * sgn[None, :]).T.copy()
    cs_b = cs_rep.astype(NPBF16)
    sn_b = sn_rep.astype(NPBF16)

    in_maps = []
    for c in range(8):
        b, g = c // 4, c % 4
        rows = slice(g * VF, (g + 1) * VF)
        wq_p = wq[rows][permH]               # [256, 1024]
        wk_p = wk[rows][permH]
        wqkT = np.concatenate([wq_p, wk_p], axis=0).T.copy()   # [1024, 512]
        wvT = wv[rows][permV].T.copy()                         # [1024, 256]
        wp_rhs = w_proj[:, rows].T.copy()                      # [256, 1024]
        xT = x[b].T.copy()                                     # [1024, 2048]
        in_maps.append({
            "xT": xT.astype(NPBF16),
            "wqkT": wqkT.astype(NPBF16),
            "wvT": wvT.astype(NPBF16),
            "wp": wp_rhs.astype(NPBF16),
            "cs": cs_b,
            "sn": sn_b,
        })
    return in_maps


def _reset_device():
    try:
        import ctypes
        import jax
        jax.devices()
        lib = ctypes.CDLL("/opt/axon/libaxon_pjrt.so")
        if hasattr(lib, "axon_reset"):
            lib.axon_reset.restype = ctypes.c_int64
            lib.axon_reset()
    except Exception:
        pass


def _get_nc():
    if "nc" not in _CACHE:
        _CACHE["nc"] = _build_nc()
    return _CACHE["nc"]


def kernel(x, freqs, w_qkv, w_proj, b_proj):
    x = np.asarray(x, dtype=np.float32)
    freqs = np.asarray(freqs, dtype=np.float32)
    w_qkv = np.asarray(w_qkv, dtype=np.float32)
    w_proj = np.asarray(w_proj, dtype=np.float32)
    b_proj = np.asarray(b_proj, dtype=np.float32)

    nc = _get_nc()
    in_maps = _make_in_maps(x, freqs, w_qkv, w_proj)
    try:
        results = bass2jax.run_bass_via_pjrt(nc, in_maps, n_cores=8)
    except Exception:
        # a previously crashed run can leave the accelerator unrecoverable;
        # reset once and retry
        _reset_device()
        results = bass2jax.run_bass_via_pjrt(nc, in_maps, n_cores=8)

    out = np.zeros((B, N, DIM), dtype=np.float32)
    for c in range(8):
        out[c // 4] += np.asarray(results[c]["out"], dtype=np.float32)
    out += b_proj[None, None, :]
    return out
